# revision 21
# baseline (speedup 1.0000x reference)
"""Blocksparse conv2d (3x3, stride 1, pad 1) on 8 Trainium2 NeuronCores.

Strategy
--------
Data-parallel over batch: 16 images -> 2 per core, identical SPMD program.

The mask zeroes whole 32x32 (cout, cin) channel blocks; the host inspects
the runtime mask and specializes the schedule: only surviving input-channel
blocks are shipped/loaded (K_used channels).  When 2*K_used <= 128 the two
images of a core are PAIRED across PE row groups (img0 on partitions
0:K_used, img1 on K_used:2*K_used) so their matmul streams run concurrently
on independent row tiles of the systolic array -- full-array throughput
without duplicating any x data (the previous replication scheme doubled the
x HBM traffic for the same PE rate).

Conv is lowered to 9 shifted matmuls accumulating in PSUM.  The host
pre-pads each image with a zero border (130x130), so every tap is a clean
2D-strided view of one flat SBUF buffer -- no edge-column fixup matmuls at
all.  The x load is chunked into 8 row-band DMAs so the matmul pipeline
starts as soon as the first band lands instead of waiting ~25us for the
full load (the old kernel's single-shot load serialized the whole kernel).

Weights are premultiplied by the mask, transposed and replicated on the
host, and DMA'd once.  Everything (x and weights) is cast fp32->bf16 inside
the SWDGE load DMAs: bf16 halves LDWEIGHTS time (fast-weight-load works for
non-fp32 dtypes) and keeps conv error ~2e-3, well under the 2e-2 gate.
Bias is fused into the PSUM->SBUF copy, alternating between the scalar and
vector engines so neither becomes the straggler.
"""

import numpy as np
from contextlib import ExitStack

import concourse.bass as bass
import concourse.tile as tile
from concourse import mybir, bacc
from concourse import bass_utils

# Problem shape (hardcoded per contract)
B, CIN, COUT, H, W = 16, 128, 128, 128, 128
KH, KW = 3, 3
BLK = 32
NCORES = 8
BPC = B // NCORES            # images per core
PH, PW = H + 2, W + 2        # host zero-padded image (130 x 130)
FLAT = PH * PW

RPW = 4                      # output rows per PSUM window (N = 512 = full bank)
NWIN = H // RPW              # 32 windows
CHUNKS = [8, 8, 8, 4, 2, 2]  # windows per output-DMA chunk (tapered tail so the
                             # final y store drains fast)
# windows per matmul group per image, per chunk.  Singleton groups up front so
# the first matmuls gate only on the first tiny x chunk; groups of 3 later
# amortize the (unhidden) LDWEIGHTS while leaving 2 spare PSUM banks so group
# g+1's matmuls overlap group g's copy-out (groups of 4 measured slower).
GROUPS = [[1, 1, 1, 2, 3], [3, 3, 2], [3, 3, 2], [3, 1], [2], [2]]

_cache = {}
_last_in_maps = None


def _round_f32r(a, mbits=10):
    """Round fp32 to `mbits` explicit mantissa bits, round-to-nearest-even
    (TF32-style).  Matches what the hardware's fp32->f32r cast does, so the
    device can consume the values via plain (cast-free) HWDGE DMAs."""
    v = np.ascontiguousarray(a, dtype=np.float32).view(np.uint32)
    shift = 23 - mbits
    bias = np.uint32((1 << (shift - 1)) - 1)
    lsb = (v >> np.uint32(shift)) & np.uint32(1)
    v = (v + bias + lsb) & np.uint32(~((1 << shift) - 1) & 0xFFFFFFFF)
    return v.view(np.float32)


def _build(n_ib, paired):
    """Build + compile the per-core SPMD program.

    n_ib:   number of surviving 32-channel input blocks (1..4)
    paired: both images share the partition dim on separate PE row groups
    """
    K_used = BLK * n_ib
    reps = 2 if paired else 1
    DK = reps * K_used
    assert DK <= 128

    nc = bacc.Bacc("TRN2", target_bir_lowering=False, debug=False)
    f32 = mybir.dt.float32
    f32r = mybir.dt.float32r

    # x and wT live in DRAM as float32r: the host pre-rounds the fp32 values
    # (TF32-style mantissa truncation, matching what the SWDGE cast would do)
    # so the fast HWDGE rings can load them with no cast pass and the BIR
    # verifier's "f32r inputs must be rounded" check is satisfied.
    x_in = nc.dram_tensor("x", [BPC, K_used, PH, PW], f32r, kind="ExternalInput").ap()
    w_in = nc.dram_tensor("wt", [DK, KH * KW, COUT], f32r, kind="ExternalInput").ap()
    b_in = nc.dram_tensor("bias", [COUT], f32, kind="ExternalInput").ap()
    y_out = nc.dram_tensor("y", [BPC, COUT, H, W], f32, kind="ExternalOutput").ap()

    # x-load chunk boundaries (padded-image rows).  Window w reads padded rows
    # 4w..4w+5, so a boundary at 4k+6 releases windows 0..k.  Fine-grained
    # early chunks let the matmul pipeline start ~3us in; coarser later ones
    # keep the DMA count (and SWDGE descriptor overhead) low.
    bounds = [0, 6, 14, 22, 30]
    while bounds[-1] + 16 < PH:
        bounds.append(bounds[-1] + 16)
    bounds.append(PH)

    with tile.TileContext(nc) as tc:
        with ExitStack() as ctx:
            singles = ctx.enter_context(tc.tile_pool(name="singles", bufs=1))
            stage_pool = ctx.enter_context(tc.tile_pool(name="ystage", bufs=4))
            psum_pool = ctx.enter_context(
                tc.tile_pool(name="psum", bufs=8, space="PSUM")
            )

            # Weights + bias on the Activation HWDGE ring so they drain in
            # parallel with the x chunks on the SP ring.  float32r is
            # bit-identical to fp32, so the load is a plain DMA with a
            # bitcast destination view -- no cast pass at all, and the
            # matmuls stream at full rate (1 col/cycle at N=512).
            wT = singles.tile([DK, KH * KW, COUT], f32r, name="wT")
            nc.scalar.dma_start(out=wT, in_=w_in)
            bias_sb = singles.tile([COUT, 1], f32, name="bias_sb")
            nc.scalar.dma_start(out=bias_sb, in_=b_in.unsqueeze(1))

            if paired:
                xb = singles.tile([DK, FLAT], f32r, name="xb")
                src = x_in.rearrange("b c h w -> (b c) (h w)")
                for k in range(len(bounds) - 1):
                    lo, hi = bounds[k] * PW, bounds[k + 1] * PW
                    nc.sync.dma_start(out=xb[:, lo:hi], in_=src[:, lo:hi])
                xbufs = [xb] * BPC
                img_base = [i * K_used for i in range(BPC)]
            else:
                xbufs, img_base = [], []
                for b in range(BPC):
                    xbi = singles.tile([K_used, FLAT], f32r, name=f"xb{b}")
                    src = x_in[b].rearrange("c h w -> c (h w)")
                    for k in range(len(bounds) - 1):
                        lo, hi = bounds[k] * PW, bounds[k + 1] * PW
                        nc.sync.dma_start(out=xbi[:, lo:hi], in_=src[:, lo:hi])
                    xbufs.append(xbi)
                    img_base.append(0)

            assert sum(CHUNKS) == NWIN
            assert [sum(g) for g in GROUPS] == CHUNKS
            c0 = 0
            for nwc, chunk_groups in zip(CHUNKS, GROUPS):
                wins = list(range(c0, c0 + nwc))
                chunk_r0 = RPW * c0
                chunk_nr = RPW * len(wins)
                c0 += nwc
                stages = [
                    stage_pool.tile(
                        [COUT, RPW * max(CHUNKS), W], f32, tag="stage",
                        name=f"st{b}_{chunk_r0}",
                    )
                    for b in range(BPC)
                ]
                g0 = 0
                for gsz in chunk_groups:
                    group = wins[g0 : g0 + gsz]
                    g0 += gsz
                    ps = {}
                    for b in range(BPC):
                        for w in group:
                            ps[(b, w)] = psum_pool.tile(
                                [128, 512], f32, tag="ps", name=f"ps{b}_{w}"
                            )
                    # tap-outer, image-inner: the two images' row groups
                    # alternate so each group's LDWEIGHTS hides behind the
                    # other group's matmuls.
                    for t in range(KH * KW):
                        dh, dw = divmod(t, KW)
                        for b in range(BPC):
                            base = img_base[b]
                            xbi = xbufs[b]
                            lhsT = wT[base : base + K_used, t, :]
                            for w in group:
                                q0 = (RPW * w + dh) * PW + dw
                                v = xbi[base : base + K_used, q0 : q0 + 1]
                                rhs = bass.AP(
                                    tensor=v.tensor,
                                    offset=v.offset,
                                    ap=[list(v.ap[0]), [PW, RPW], [1, W]],
                                )
                                nc.tensor.matmul(
                                    ps[(b, w)][:, : RPW * W],
                                    lhsT,
                                    rhs,
                                    start=(t == 0),
                                    stop=(t == KH * KW - 1),
                                )
                    # copy-out with fused bias, split across ACT and DVE
                    for b in range(BPC):
                        for w in group:
                            r0 = RPW * w
                            ps_v = ps[(b, w)][:, : RPW * W].rearrange(
                                "p (r s) -> p r s", s=W
                            )
                            dst = stages[b][:, r0 - chunk_r0 : r0 - chunk_r0 + RPW, :]
                            if (w + b) % 2 == 0:
                                nc.scalar.activation(
                                    out=dst,
                                    in_=ps_v,
                                    func=mybir.ActivationFunctionType.Identity,
                                    bias=bias_sb,
                                    scale=1.0,
                                )
                            else:
                                nc.vector.tensor_scalar_add(
                                    out=dst, in0=ps_v, scalar1=bias_sb
                                )
                # alternate the two HWDGE rings (SP / Activation) so the small
                # final stores don't queue behind every earlier large store
                for b in range(BPC):
                    eng = nc.sync if b == 0 else nc.scalar
                    eng.dma_start(
                        out=y_out[b][:, chunk_r0 : chunk_r0 + chunk_nr, :],
                        in_=stages[b][:, :chunk_nr, :],
                    )

    nc.compile()
    return nc


def kernel(x, weight, bias, mask):
    x = np.ascontiguousarray(np.asarray(x, dtype=np.float32))
    weight = np.asarray(weight, dtype=np.float32)
    bias = np.ascontiguousarray(np.asarray(bias, dtype=np.float32))
    mask = np.asarray(mask, dtype=np.float32)

    # --- host-side schedule specialization from the runtime mask ----------
    wm = weight * mask
    blk_any = (
        np.abs(wm).reshape(COUT, CIN // BLK, BLK, KH, KW).sum(axis=(0, 2, 3, 4)) > 0
    )
    used_ibs = [ib for ib in range(CIN // BLK) if blk_any[ib]] or [0]
    n_ib = len(used_ibs)
    K_used = BLK * n_ib
    paired = (BPC == 2) and (2 * K_used <= 128)
    reps = 2 if paired else 1

    used_ch = np.concatenate(
        [np.arange(ib * BLK, (ib + 1) * BLK) for ib in used_ibs]
    )

    key = (n_ib, paired)
    if key not in _cache:
        _cache[key] = _build(n_ib, paired)
    nc = _cache[key]

    # wT[rep*K_used + c, t, o] = (w*m)[o, used_ch[c], tap t]
    wT = wm[:, used_ch].reshape(COUT, K_used, KH * KW).transpose(1, 2, 0)
    wT = _round_f32r(
        np.ascontiguousarray(np.concatenate([wT] * reps, axis=0), dtype=np.float32)
    )

    # zero-padded x (130x130) restricted to the used channels, f32r-rounded
    xp = np.zeros((B, K_used, PH, PW), dtype=np.float32)
    xp[:, :, 1 : H + 1, 1 : W + 1] = _round_f32r(x[:, used_ch])

    in_maps = []
    for core in range(NCORES):
        xs = np.ascontiguousarray(xp[core * BPC : (core + 1) * BPC])
        in_maps.append({"x": xs, "wt": wT, "bias": bias})

    global _last_in_maps
    _last_in_maps = in_maps

    res = bass_utils.run_bass_kernel_spmd(nc, in_maps, core_ids=list(range(NCORES)))
    y = np.concatenate([res.results[c]["y"] for c in range(NCORES)], axis=0)
    return y


# revision 28
# speedup vs baseline: 1.0690x; 1.0690x over previous
"""Blocksparse conv2d (3x3, stride 1, pad 1) on 8 Trainium2 NeuronCores.

Strategy
--------
Data-parallel over batch: 16 images -> 2 per core, identical SPMD program.

The mask zeroes whole 32x32 (cout, cin) channel blocks; the host inspects
the runtime mask and specializes the schedule: only surviving input-channel
blocks are shipped/loaded (K_used channels).  When 2*K_used <= 128 the two
images of a core are PAIRED across PE row groups (img0 on partitions
0:K_used, img1 on K_used:2*K_used) so their matmul streams run concurrently
on independent row tiles of the systolic array -- full-array throughput
without duplicating any x data (the previous replication scheme doubled the
x HBM traffic for the same PE rate).

Conv is lowered to 9 shifted matmuls accumulating in PSUM.  The host
pre-pads each image with a zero border (130x130), so every tap is a clean
2D-strided view of one flat SBUF buffer -- no edge-column fixup matmuls at
all.  The x load is chunked into 8 row-band DMAs so the matmul pipeline
starts as soon as the first band lands instead of waiting ~25us for the
full load (the old kernel's single-shot load serialized the whole kernel).

Weights are premultiplied by the mask, transposed and replicated on the
host, and DMA'd once.  Everything (x and weights) is cast fp32->bf16 inside
the SWDGE load DMAs: bf16 halves LDWEIGHTS time (fast-weight-load works for
non-fp32 dtypes) and keeps conv error ~2e-3, well under the 2e-2 gate.
Bias is fused into the PSUM->SBUF copy, alternating between the scalar and
vector engines so neither becomes the straggler.
"""

import numpy as np
from contextlib import ExitStack

import concourse.bass as bass
import concourse.tile as tile
from concourse import mybir, bacc
from concourse import bass_utils

# Problem shape (hardcoded per contract)
B, CIN, COUT, H, W = 16, 128, 128, 128, 128
KH, KW = 3, 3
BLK = 32
NCORES = 8
BPC = B // NCORES            # images per core
PH, PW = H + 2, W + 2        # host zero-padded image (130 x 130)
FLAT = PH * PW

RPW = 4                      # output rows per PSUM window (N = 512 = full bank)
NWIN = H // RPW              # 32 windows
CHUNKS = [8, 8, 8, 4, 2, 2]  # windows per output-DMA chunk (tapered tail so the
                             # final y store drains fast)
# windows per matmul group per image, per chunk.  Singleton groups up front so
# the first matmuls gate only on the first tiny x chunk; groups of 3 later
# amortize the (unhidden) LDWEIGHTS while leaving 2 spare PSUM banks so group
# g+1's matmuls overlap group g's copy-out (groups of 4 measured slower).
GROUPS = [[1, 1, 1, 2, 3], [3, 3, 2], [3, 3, 2], [3, 1], [2], [2]]

_cache = {}
_last_in_maps = None


def _build(n_ib, paired):
    """Build + compile the per-core SPMD program.

    n_ib:   number of surviving 32-channel input blocks (1..4)
    paired: both images share the partition dim on separate PE row groups
    """
    K_used = BLK * n_ib
    reps = 2 if paired else 1
    DK = reps * K_used
    assert DK <= 128

    nc = bacc.Bacc("TRN2", target_bir_lowering=False, debug=False)
    f32 = mybir.dt.float32
    bf16 = mybir.dt.bfloat16

    x_in = nc.dram_tensor("x", [BPC, K_used, PH, PW], f32, kind="ExternalInput").ap()
    w_in = nc.dram_tensor("wt", [DK, KH * KW, COUT], f32, kind="ExternalInput").ap()
    b_in = nc.dram_tensor("bias", [COUT], f32, kind="ExternalInput").ap()
    y_out = nc.dram_tensor("y", [BPC, COUT, H, W], f32, kind="ExternalOutput").ap()

    # x-load chunk boundaries (padded-image rows).  Window w reads padded rows
    # 4w..4w+5, so a boundary at 4k+6 releases windows 0..k.  Fine-grained
    # early chunks let the matmul pipeline start ~3us in; coarser later ones
    # keep the DMA count (and SWDGE descriptor overhead) low.
    bounds = [0, 6, 14, 22, 30]
    while bounds[-1] + 16 < PH:
        bounds.append(bounds[-1] + 16)
    bounds.append(PH)

    with tile.TileContext(nc) as tc:
        with ExitStack() as ctx:
            singles = ctx.enter_context(tc.tile_pool(name="singles", bufs=1))
            stage_pool = ctx.enter_context(tc.tile_pool(name="ystage", bufs=4))
            psum_pool = ctx.enter_context(
                tc.tile_pool(name="psum", bufs=8, space="PSUM")
            )

            # Weights + bias on the Activation HWDGE ring so they drain in
            # parallel with the x chunks on the SP ring.  Loads are plain fp32
            # HWDGE DMAs (fast trigger, no SWDGE descriptor serialization);
            # the fp32 -> bf16 rounding runs as chunked DVE casts.  bf16
            # matmuls stream at the full 1 col/cycle even with the strided
            # rhs view (f32r measured ~2 cycles/col at N=512 there).
            wT_f32 = singles.tile([DK, KH * KW, COUT], f32, name="wT_f32")
            wT = singles.tile([DK, KH * KW, COUT], bf16, name="wT")
            nc.scalar.dma_start(out=wT_f32, in_=w_in)
            nc.vector.tensor_copy(out=wT, in_=wT_f32)
            bias_sb = singles.tile([COUT, 1], f32, name="bias_sb")
            nc.scalar.dma_start(out=bias_sb, in_=b_in.unsqueeze(1))

            def load_chunks(xstage, xbt, src):
                for k in range(len(bounds) - 1):
                    lo, hi = bounds[k] * PW, bounds[k + 1] * PW
                    nc.sync.dma_start(out=xstage[:, lo:hi], in_=src[:, lo:hi])
                    nc.vector.tensor_copy(out=xbt[:, lo:hi], in_=xstage[:, lo:hi])

            if paired:
                xstage = singles.tile([DK, FLAT], f32, name="xstage")
                xb = singles.tile([DK, FLAT], bf16, name="xb")
                load_chunks(xstage, xb, x_in.rearrange("b c h w -> (b c) (h w)"))
                xbufs = [xb] * BPC
                img_base = [i * K_used for i in range(BPC)]
            else:
                # fallback (3-4 surviving input blocks): SWDGE cast loads --
                # no room for fp32 staging buffers for both images
                xbufs, img_base = [], []
                for b in range(BPC):
                    xbi = singles.tile([K_used, FLAT], bf16, name=f"xb{b}")
                    src = x_in[b].rearrange("c h w -> c (h w)")
                    for k in range(len(bounds) - 1):
                        lo, hi = bounds[k] * PW, bounds[k + 1] * PW
                        nc.gpsimd.dma_start(out=xbi[:, lo:hi], in_=src[:, lo:hi])
                    xbufs.append(xbi)
                    img_base.append(0)

            assert sum(CHUNKS) == NWIN
            assert [sum(g) for g in GROUPS] == CHUNKS
            c0 = 0
            for nwc, chunk_groups in zip(CHUNKS, GROUPS):
                wins = list(range(c0, c0 + nwc))
                chunk_r0 = RPW * c0
                chunk_nr = RPW * len(wins)
                c0 += nwc
                stages = [
                    stage_pool.tile(
                        [COUT, RPW * max(CHUNKS), W], f32, tag="stage",
                        name=f"st{b}_{chunk_r0}",
                    )
                    for b in range(BPC)
                ]
                g0 = 0
                for gsz in chunk_groups:
                    group = wins[g0 : g0 + gsz]
                    g0 += gsz
                    ps = {}
                    for b in range(BPC):
                        for w in group:
                            ps[(b, w)] = psum_pool.tile(
                                [128, 512], f32, tag="ps", name=f"ps{b}_{w}"
                            )
                    # tap-outer, image-inner: the two images' row groups
                    # alternate so each group's LDWEIGHTS hides behind the
                    # other group's matmuls.
                    for t in range(KH * KW):
                        dh, dw = divmod(t, KW)
                        for b in range(BPC):
                            base = img_base[b]
                            xbi = xbufs[b]
                            lhsT = wT[base : base + K_used, t, :]
                            for w in group:
                                q0 = (RPW * w + dh) * PW + dw
                                v = xbi[base : base + K_used, q0 : q0 + 1]
                                rhs = bass.AP(
                                    tensor=v.tensor,
                                    offset=v.offset,
                                    ap=[list(v.ap[0]), [PW, RPW], [1, W]],
                                )
                                nc.tensor.matmul(
                                    ps[(b, w)][:, : RPW * W],
                                    lhsT,
                                    rhs,
                                    start=(t == 0),
                                    stop=(t == KH * KW - 1),
                                )
                    # copy-out with fused bias.  All on ACT: the vector engine
                    # runs the fp32->bf16 x casts, and a copy queued behind a
                    # not-yet-ready cast would stall PSUM bank recycling.
                    for b in range(BPC):
                        for w in group:
                            r0 = RPW * w
                            ps_v = ps[(b, w)][:, : RPW * W].rearrange(
                                "p (r s) -> p r s", s=W
                            )
                            dst = stages[b][:, r0 - chunk_r0 : r0 - chunk_r0 + RPW, :]
                            nc.scalar.activation(
                                out=dst,
                                in_=ps_v,
                                func=mybir.ActivationFunctionType.Identity,
                                bias=bias_sb,
                                scale=1.0,
                            )
                # alternate the two HWDGE rings (SP / Activation) so the small
                # final stores don't queue behind every earlier large store
                for b in range(BPC):
                    eng = nc.sync if b == 0 else nc.scalar
                    eng.dma_start(
                        out=y_out[b][:, chunk_r0 : chunk_r0 + chunk_nr, :],
                        in_=stages[b][:, :chunk_nr, :],
                    )

    nc.compile()
    return nc


def kernel(x, weight, bias, mask):
    x = np.ascontiguousarray(np.asarray(x, dtype=np.float32))
    weight = np.asarray(weight, dtype=np.float32)
    bias = np.ascontiguousarray(np.asarray(bias, dtype=np.float32))
    mask = np.asarray(mask, dtype=np.float32)

    # --- host-side schedule specialization from the runtime mask ----------
    wm = weight * mask
    blk_any = (
        np.abs(wm).reshape(COUT, CIN // BLK, BLK, KH, KW).sum(axis=(0, 2, 3, 4)) > 0
    )
    used_ibs = [ib for ib in range(CIN // BLK) if blk_any[ib]] or [0]
    n_ib = len(used_ibs)
    K_used = BLK * n_ib
    paired = (BPC == 2) and (2 * K_used <= 128)
    reps = 2 if paired else 1

    used_ch = np.concatenate(
        [np.arange(ib * BLK, (ib + 1) * BLK) for ib in used_ibs]
    )

    key = (n_ib, paired)
    if key not in _cache:
        _cache[key] = _build(n_ib, paired)
    nc = _cache[key]

    # wT[rep*K_used + c, t, o] = (w*m)[o, used_ch[c], tap t]
    wT = wm[:, used_ch].reshape(COUT, K_used, KH * KW).transpose(1, 2, 0)
    wT = np.ascontiguousarray(
        np.concatenate([wT] * reps, axis=0), dtype=np.float32
    )

    # zero-padded x (130x130) restricted to the used channels
    xp = np.zeros((B, K_used, PH, PW), dtype=np.float32)
    xp[:, :, 1 : H + 1, 1 : W + 1] = x[:, used_ch]

    in_maps = []
    for core in range(NCORES):
        xs = np.ascontiguousarray(xp[core * BPC : (core + 1) * BPC])
        in_maps.append({"x": xs, "wt": wT, "bias": bias})

    global _last_in_maps
    _last_in_maps = in_maps

    res = bass_utils.run_bass_kernel_spmd(nc, in_maps, core_ids=list(range(NCORES)))
    y = np.concatenate([res.results[c]["y"] for c in range(NCORES)], axis=0)
    return y
